# revision 31
# baseline (speedup 1.0000x reference)
"""Trainium2 kernel for nn_BlockLinear: gather -> per-block GEMM -> scatter-add.

Key insight: the whole op is linear in x, so gather/einsum/scatter fold into a
single dense GEMM  out[t, o] = sum_k x[t, k] * Wfull[k, o] + bias[o]  where
Wfull[k, o] = sum_{n,i,j} [input_indices[n,i]==k][output_indices[n,j]==o] * W[n,j,i].

Wfull is built on host (bincount scatter-add, exact fp64 accumulation), then the
GEMM runs on 8 NeuronCores, sharded 2D: 4 token groups x 2 out-feature groups.
Matmuls run in bf16 (same PE rate as fp32r, half the HBM/DMA traffic, so the
x-streaming warmup phase is no longer DMA-starved); accumulation is fp32 in
PSUM and the bias add + output stay full fp32.
"""

import numpy as np
import ml_dtypes
import concourse.bacc as bacc
import concourse.mybir as mybir
import concourse.tile as tile
from concourse.bass_utils import run_bass_kernel_spmd

# problem shapes (hardcoded per contract)
B, S = 2, 2048
IN_FEATURES = 4096
OUT_FEATURES = 4096
NTOKENS = B * S                  # 4096

NCORES = 8
TG, OG = 4, 2                    # token groups x out-feature groups
T = NTOKENS // TG                # 1024 tokens per core
O = OUT_FEATURES // OG           # 2048 out features per core
P = 128
KT = IN_FEATURES // P            # 32 contraction tiles
OT = O // P                      # 16 out-feature tiles per core
NTOK = 512                       # moving free dim per matmul
TB = T // NTOK                   # 2 token blocks per core

BF16 = mybir.dt.bfloat16
F32 = mybir.dt.float32

# knobs for test.py
TRACE = False
LAST_RESULTS = None

WCHUNK = 8        # k-tiles per W DMA (2KB/partition contiguous)
WBUFS = 16        # W chunk pool bufs (4 o-groups in flight)
NDUMMY = 8        # PE HAM warmup matmuls; bridge until first x/W DMAs land


def build_nc(repeats: int = 1):
    nc = bacc.Bacc()
    # xT slab pairs: [j][128, 2, TB*NTOK] bf16 (4KB/partition per DMA)
    xw = nc.dram_tensor(
        "xw", [KT // 2, P, 2, TB * NTOK], BF16, kind="ExternalInput"
    )
    # W per o-group: [o][128, KT, 128] bf16 -- 8KB/partition contiguous, so a
    # whole group loads in ONE 1MB DMA (steady phase); the warmup slices it
    # into small consumption-ordered pieces
    wbig = nc.dram_tensor("wbig", [OT, P, KT, P], BF16, kind="ExternalInput")
    # bias in o-partition layout: [128, OT]
    bo = nc.dram_tensor("bo", [P, OT], F32, kind="ExternalInput")
    # output in bf16: halves the exposed final drain DMA flight and doubles
    # DVE drain throughput; host converts back to f32 (adds ~1.1e-3 rel err
    # in quadrature -- gate is 2e-2)
    out = nc.dram_tensor("out", [OT, TB, P, NTOK], BF16, kind="ExternalOutput")

    NWARM = 4  # o-groups processed k-major while the xT stream arrives

    with tile.TileContext(nc) as tc:
        with (
            tc.tile_pool(name="xw_sb", bufs=1) as xw_sb,
            tc.tile_pool(name="w_sb", bufs=WBUFS) as w_sb,
            tc.tile_pool(name="wg_sb", bufs=3) as wg_sb,
            tc.tile_pool(name="o_sb", bufs=6) as o_sb,
            tc.tile_pool(name="ps", bufs=8, space="PSUM") as ps,
        ):
            bo_t = xw_sb.tile([P, OT], F32, tag="bo")

            # PE HAM warmup: dummy matmuls on memset data fill the dead time
            # while the first DMAs land, so real matmuls start at 2.4 GHz
            dummy_sb = xw_sb.tile([P, NTOK], BF16, tag="dummy")
            nc.vector.memset(dummy_sb.bitcast(F32), 0.0)
            ps_d = ps.tile([P, NTOK], F32, tag="ps", name="ps_dummy")
            for _ in range(NDUMMY):
                nc.tensor.matmul(
                    ps_d, dummy_sb[:, :P], dummy_sb, start=True, stop=True
                )

            wts = {}
            wgs = {}

            def load_w(o, rep):
                # one 1MB DMA for the whole o-group: 8KB/partition contiguous
                # (near-peak DMA efficiency) and a single trigger, so the
                # 8-semaphore-lane DMA trigger pipeline never backs up.
                # All on the sync ring: FIFO order queues them BEHIND the
                # warmup stream, so they cannot steal its bandwidth.
                wg = wg_sb.tile([P, KT, P], BF16, tag="wg", name=f"wg_{rep}_{o}")
                nc.sync.dma_start(out=wg, in_=wbig[o])
                wgs[o] = wg

            # Warmup DMA schedule: the warmup consumes, per k-tile, the 4
            # o-groups' weight tiles (4x32KB) + one x slab (256KB) every
            # ~1.7us, i.e. x needs ~148GB/s while W needs ~72GB/s.  Rings
            # drain FIFO per queue with packet round-robin BETWEEN queues, so
            # the share each stream gets is set by how many rings carry it:
            # x pairs alternate over BOTH HWDGE rings (sync+scalar -> ~2/3
            # share), while warmup W rides the gpsimd SWDGE ring, all issued
            # in exact consumption order.  (One shared ring for x and W
            # leaves x only ~1.6us of margin and the PE stalls at k~10-12,
            # re-throttling HAM.)
            xw_t = {}
            KC = KT // WCHUNK
            SUBW = 4  # k-tiles per warmup W sub-DMA (1KB/partition descs)

            def rhs(k, tb):
                return xw_t[k // 2][:, k % 2, tb * NTOK : (tb + 1) * NTOK]

            def load_x(j, split=False):
                # one DMA per PAIR of k-tiles (512KB, 4KB/partition)
                t = xw_sb.tile([P, 2, TB * NTOK], BF16, tag=f"xw_{j}")
                if split:
                    # quarter the first pair so the k=0/tb=0 piece (128KB)
                    # unblocks the first real matmul early
                    for a in range(2):
                        nc.sync.dma_start(
                            out=t[:, a, :NTOK], in_=xw[j][:, a, :NTOK]
                        )
                        nc.sync.dma_start(
                            out=t[:, a, NTOK:], in_=xw[j][:, a, NTOK:]
                        )
                else:
                    nc.sync.dma_start(out=t, in_=xw[j])
                xw_t[j] = t

            # The ENTIRE warmup stream (W pieces + x pairs) rides the sync
            # HWDGE ring alone, interleaved in exact consumption order: a
            # single active queue gets the full fabric rate (two active
            # queues round-robin ~50/50 regardless of need), and FIFO order
            # guarantees first-needed bytes arrive first.  The scalar ring
            # carries only the out-drain DMAs, which start ~60us in.
            def w_sub(s):
                for o in range(NWARM):
                    nc.sync.dma_start(
                        out=wts[o, 0][:, s * SUBW : (s + 1) * SUBW],
                        in_=wbig[o][:, s * SUBW : (s + 1) * SUBW],
                    )

            def w_kc(kc):
                for o in range(NWARM):
                    wt = w_sb.tile(
                        [P, WCHUNK, P], BF16, tag="wt", name=f"wt_0_{o}_{kc}"
                    )
                    nc.sync.dma_start(
                        out=wt, in_=wbig[o][:, kc * WCHUNK : (kc + 1) * WCHUNK]
                    )
                    wts[o, kc] = wt

            for o in range(NWARM):
                wts[o, 0] = w_sb.tile(
                    [P, WCHUNK, P], BF16, tag="wt", name=f"wt_0_{o}_0"
                )
            w_sub(0)                      # weights for k=0-3
            load_x(0, split=True)         # x k=0,1 (quartered)
            load_x(1)                     # x k=2,3
            w_sub(1)                      # weights for k=4-7
            load_x(2)
            load_x(3)
            w_kc(1)                       # weights for k=8-15
            for j in range(4, 8):
                load_x(j)
            w_kc(2)                       # weights for k=16-23
            for j in range(8, 12):
                load_x(j)
            w_kc(3)                       # weights for k=24-31
            for j in range(12, 16):
                load_x(j)
            # bias is only needed by the drains, ~60us in
            nc.sync.dma_start(out=bo_t, in_=bo[:, :])

            def drain(o, tb, psum, split_dma=False):
                o_t = o_sb.tile([P, NTOK], BF16, tag="ot", name=f"ot_{o}_{tb}")
                # psum -> sbuf with per-partition bias add; alternate engines
                # so consecutive drains run in parallel
                if (o * TB + tb) % 2 == 0:
                    nc.scalar.add(o_t, psum, bo_t[:, o : o + 1])
                else:
                    nc.vector.tensor_scalar_add(o_t, psum, bo_t[:, o : o + 1])
                if split_dma:
                    # final drain: halve the exposed DMA flight time by
                    # shipping the two halves on independent queues
                    h = NTOK // 2
                    nc.scalar.dma_start(out=out[o, tb, :, :h], in_=o_t[:, :h])
                    nc.sync.dma_start(out=out[o, tb, :, h:], in_=o_t[:, h:])
                else:
                    nc.scalar.dma_start(out=out[o, tb, :, :], in_=o_t)

            def mm_group(o, rep, last=False):
                psums = {
                    tb: ps.tile([P, NTOK], F32, tag="ps", name=f"ps_{rep}_{o}_{tb}")
                    for tb in range(TB)
                }
                wg = wgs[o]
                if last:
                    # final group: k-major per token block, so tb=0's drain +
                    # out DMA hide under tb=1's matmuls and only one drain is
                    # exposed after the last matmul
                    for tb in range(TB):
                        for k in range(KT):
                            nc.tensor.matmul(
                                psums[tb],
                                wg[:, k],
                                rhs(k, tb),
                                start=(k == 0),
                                stop=(k == KT - 1),
                            )
                        drain(o, tb, psums[tb], split_dma=(tb == TB - 1))
                    return
                for k in range(KT):
                    for tb in range(TB):
                        nc.tensor.matmul(
                            psums[tb],
                            wg[:, k],
                            rhs(k, tb),
                            start=(k == 0),
                            stop=(k == KT - 1),
                        )
                for tb in range(TB):
                    drain(o, tb, psums[tb])

            for _rep in range(repeats):
                if _rep == 0:
                    # warmup phase: k-major over NWARM o-groups x TB token
                    # blocks (all 8 psum banks) -> 8 matmuls per arriving
                    # xT k-slab, keeping the PE busy while xT streams in
                    psums = {
                        (o, tb): ps.tile(
                            [P, NTOK], F32, tag="ps", name=f"psw_{o}_{tb}"
                        )
                        for o in range(NWARM)
                        for tb in range(TB)
                    }
                    # Slow-start: the first SLOWK k-tiles run as N=256
                    # quarter-matmuls -- same FLOPs and ~same wall time, but
                    # the PE consumes x/W at HALF the arrival rate the DMA
                    # must sustain, building up a delivery lead.  (At full
                    # rate the first ~30us need ~245GB/s, right at fabric
                    # capacity, and any jitter stalls the PE + re-throttles
                    # HAM.)  start=True only on the very first MM touching
                    # each PSUM bank (it clears the whole bank's has_written
                    # bits); the q=1 quarter relies on cleared bits to
                    # overwrite, then accumulates normally.
                    SLOWK = 12
                    NQ = NTOK // 2
                    for k in range(KT - WCHUNK):
                        for o in range(NWARM):
                            lhsT = wts[o, k // WCHUNK][:, k % WCHUNK]
                            for tb in range(TB):
                                if k < SLOWK:
                                    for q in range(2):
                                        nc.tensor.matmul(
                                            psums[o, tb][
                                                :, q * NQ : (q + 1) * NQ
                                            ],
                                            lhsT,
                                            xw_t[k // 2][
                                                :,
                                                k % 2,
                                                tb * NTOK
                                                + q * NQ : tb * NTOK
                                                + (q + 1) * NQ,
                                            ],
                                            start=(k == 0 and q == 0),
                                            stop=False,
                                        )
                                else:
                                    nc.tensor.matmul(
                                        psums[o, tb],
                                        lhsT,
                                        rhs(k, tb),
                                        start=False,
                                        stop=False,
                                    )
                    # last k-window o-major with immediate drains, so psum
                    # banks free one o-group at a time and the steady phase
                    # starts while the rest of the warmup finishes
                    for o in range(NWARM):
                        for k in range(KT - WCHUNK, KT):
                            lhsT = wts[o, k // WCHUNK][:, k % WCHUNK]
                            for tb in range(TB):
                                nc.tensor.matmul(
                                    psums[o, tb],
                                    lhsT,
                                    rhs(k, tb),
                                    start=False,
                                    stop=(k == KT - 1),
                                )
                        for tb in range(TB):
                            drain(o, tb, psums[o, tb])
                    o_start = NWARM
                else:
                    o_start = 0
                for o in range(o_start, OT):
                    load_w(o, _rep)
                    mm_group(o, _rep, last=(_rep == repeats - 1 and o == OT - 1))
    nc.finalize()
    return nc


_NC = None


def _get_nc():
    global _NC
    if _NC is None:
        _NC = build_nc()
    return _NC


def _build_wfull(weights, input_indices, output_indices):
    """Wfull[k, o] = sum over blocks/dups of weights[n, j, i]."""
    ii = np.asarray(input_indices).astype(np.int64)     # [NBLK, BI]
    oi = np.asarray(output_indices).astype(np.int64)    # [NBLK, BO]
    w = np.asarray(weights, dtype=np.float64)           # [NBLK, BO, BI]
    flat = (ii[:, :, None] * OUT_FEATURES + oi[:, None, :]).ravel()  # [n, i, j]
    vals = np.ascontiguousarray(np.swapaxes(w, 1, 2)).ravel()        # [n, i, j]
    wfull = np.bincount(flat, weights=vals, minlength=IN_FEATURES * OUT_FEATURES)
    return wfull.reshape(IN_FEATURES, OUT_FEATURES)


def prepare_in_maps(x, weights, bias, input_indices, output_indices):
    x = np.asarray(x, dtype=np.float32)
    bias = np.asarray(bias, dtype=np.float32)

    wfull = _build_wfull(weights, input_indices, output_indices).astype(
        ml_dtypes.bfloat16
    )
    xr = x.reshape(NTOKENS, IN_FEATURES).astype(ml_dtypes.bfloat16)

    in_maps = []
    for c in range(NCORES):
        tg, og = divmod(c, OG)
        xT = np.ascontiguousarray(xr[tg * T : (tg + 1) * T, :].T)   # [K, T]
        # pair layout [KT/2, P(k), 2, T]: 4KB/partition contiguous per pair
        xwm = np.ascontiguousarray(
            xT.reshape(KT // 2, 2, P, T).transpose(0, 2, 1, 3)
        )
        # [K, O/2] -> [OT, P(k-in-tile), KT, P(o)]: 8KB/partition contiguous
        wr = np.ascontiguousarray(
            wfull[:, og * O : (og + 1) * O]
            .reshape(KT, P, OT, P)
            .transpose(2, 1, 0, 3)
        )
        # bias in o-partition layout [128, OT]; full fp32 (added exactly on ACT)
        bo = np.ascontiguousarray(
            bias[og * O : (og + 1) * O].reshape(OT, P).T
        )
        in_maps.append({"xw": xwm, "wbig": wr, "bo": bo})
    return in_maps


def assemble_output(core_outs):
    full = np.empty((NTOKENS, OUT_FEATURES), np.float32)
    for c in range(NCORES):
        tg, og = divmod(c, OG)
        o4 = np.asarray(core_outs[c]).astype(np.float32)  # [OT, TB, P, NTOK]
        blk = o4.transpose(1, 3, 0, 2).reshape(T, O)     # [t, o]
        full[tg * T : (tg + 1) * T, og * O : (og + 1) * O] = blk
    return full.reshape(B, S, OUT_FEATURES)


def kernel(x, weights, bias, input_indices, output_indices):
    global LAST_RESULTS
    in_maps = prepare_in_maps(x, weights, bias, input_indices, output_indices)
    nc = _get_nc()
    res = run_bass_kernel_spmd(nc, in_maps, list(range(NCORES)))
    LAST_RESULTS = res
    return assemble_output([res.results[c]["out"] for c in range(NCORES)])
